# revision 19
# baseline (speedup 1.0000x reference)
"""MiniMax-M2 sparse MoE block on 8 Trainium2 NeuronCores.

Expert-parallel with host-side dispatch (per the sharding hint): the host
computes the tiny router (512x16 GEMM + sigmoid + top-2), gathers each
expert's tokens, and ships per-core shards; each core runs its 2 local
experts' SwiGLU MLPs on just their routed tokens (capacity C=128 slots,
seed-stable max load ~84) and the host scatter-adds the expert outputs
back with the combine weights (the unshard step).

Weight prep happens at shard time on the host: block-dequant (the
reference's `_dequant`), cast to bf16, and transpose into the
contraction-major layout the PE needs. The device then streams ~20 MB of
bf16 weights per core -- the memory roofline this kernel targets -- and
does all the heavy math:

  - up/gate: stationary xT chunks [h,128 tok], moving w1T/w3T [h, I]
    slices, PSUM-accumulated over 16 h-blocks -> g,u [tok, I].
  - SwiGLU on ACT/DVE, PE-transpose of a -> aT [i, tok].
  - down: stationary aT chunks, moving w2T [i, H] slices, accumulated
    over 6 i-blocks -> y [tok, H], stored bf16.

All loads are a handful of large HWDGE DMAs (contiguous per partition).
"""

import os
import sys
import numpy as np

for _p in ("/opt/trn_rl_repo", "/root/.axon_site/_ro/trn_rl_repo"):
    if os.path.isdir(_p) and _p not in sys.path:
        sys.path.insert(0, _p)
        break

T, H, I, E = 512, 2048, 768, 16
NCORES, EPC = 8, 2
P = 128
HB, IB, TC = H // P, I // P, T // P      # 16, 6, 4
C = 96                                    # token capacity per expert
# (Binomial(512, 1/8) loads: mean 64, sd 7.5 -- 96 is a >4-sigma cap.
# kernel() falls back to a C=128 build if an expert ever exceeds it.)

_CACHE = {}


def _bf16():
    import ml_dtypes
    return ml_dtypes.bfloat16


def route(x, gate_w):
    """Host router: exact reference semantics (sigmoid + top-2 + sum-norm).

    Returns (tids, cw): per expert, the token ids routed to it and their
    combine weights.
    """
    logits = x.astype(np.float32) @ gate_w.T.astype(np.float32)
    s = (1.0 / (1.0 + np.exp(-logits))).astype(np.float32)
    order = np.argsort(-s, axis=-1, kind="stable")   # ties: lower idx first
    top2 = order[:, :2]
    denom = (np.take_along_axis(s, top2, axis=1).sum(axis=1)
             .astype(np.float32))
    tids, cw = [], []
    for e in range(E):
        hit = (top2 == e).any(axis=1)
        tid = np.nonzero(hit)[0]
        tids.append(tid)
        cw.append((s[tid, e] / denom[tid]).astype(np.float32))
    return tids, cw


def _emit_body(nc, mybir, pools, dram, ident):
    f32 = mybir.dt.float32
    bf16 = mybir.dt.bfloat16
    AF = mybir.ActivationFunctionType
    OP = mybir.AluOpType
    (xtp, w13p, w2p, ap, atp, yp, ps) = pools
    (xt_d, w1t_d, w3t_d, w2t_d, y_d) = dram

    for e in range(EPC):
        # ---- loads: chunk-contiguous HWDGE DMAs (128 descriptors each),
        # spread across both rings (sync + scalar) ----
        xt = xtp.tile([P, HB, C], bf16, tag="xt", name="xt")
        nc.sync.dma_start(xt[:], xt_d[e])
        w1t, w3t = [], []
        for k in range(4):
            t1 = w13p.tile([P, 4, I], bf16, tag="w1t", name="w1t", bufs=8)
            nc.sync.dma_start(t1[:], w1t_d[e, k])
            w1t.append(t1)
            t3 = w13p.tile([P, 4, I], bf16, tag="w3t", name="w3t", bufs=8)
            nc.scalar.dma_start(t3[:], w3t_d[e, k])
            w3t.append(t3)
        w2t = []
        for hc in range(4):
            t2 = w2p.tile([P, IB, 512], bf16, tag="w2t", name="w2t", bufs=8)
            (nc.sync if hc % 2 == 0 else nc.scalar).dma_start(
                t2[:], w2t_d[e, hc])
            w2t.append(t2)

        # ---- up/gate: psum [tok, i] accumulated over 16 h-blocks.
        # lo-half pass first, then hi-half: the lo SwiGLU chain overlaps
        # the hi matmul pass. ----
        asb = ap.tile([C, I], f32, tag="a", name="a")
        at = atp.tile([P, IB, C], bf16, tag="at", name="at")
        for (lo, hi) in ((0, 512), (512, 768)):
            w = hi - lo
            pg = ps.tile([C, w], f32, tag="pg%d" % lo, name="pg", bufs=1)
            pu = ps.tile([C, w], f32, tag="pu%d" % lo, name="pu", bufs=1)
            for hb in range(HB):
                st, sp = (hb == 0), (hb == HB - 1)
                k, j = hb // 4, hb % 4
                nc.tensor.matmul(pg[:], xt[:, hb, :], w1t[k][:, j, lo:hi],
                                 start=st, stop=sp)
                nc.tensor.matmul(pu[:], xt[:, hb, :], w3t[k][:, j, lo:hi],
                                 start=st, stop=sp)
            # SwiGLU on this half: a = silu(g) * u
            sg = ap.tile([C, w], bf16, tag="sg%d" % lo, name="sg")
            nc.scalar.activation(sg[:], pg[:], AF.Sigmoid)
            xs = ap.tile([C, w], f32, tag="xs%d" % lo, name="xs")
            nc.vector.tensor_tensor(out=xs[:], in0=sg[:], in1=pg[:],
                                    op=OP.mult)
            nc.vector.tensor_tensor(out=asb[:, lo:hi], in0=xs[:], in1=pu[:],
                                    op=OP.mult)
            # transpose finished a-chunks -> aT [i, tok] bf16 via PE
            for ic in range(lo // P, hi // P):
                pt = ps.tile([P, C], f32, tag="pt", name="pt", bufs=2)
                nc.tensor.transpose(pt[:], asb[:, ic * P:(ic + 1) * P],
                                    ident[0:C, 0:C])
                nc.vector.tensor_copy(at[:, ic, :], pt[:])

        # ---- down: psum [tok, h-chunk] accumulated over 6 i-blocks ----
        for hc in range(4):
            py = ps.tile([C, 512], f32, tag="py", name="py", bufs=2)
            for ib in range(IB):
                nc.tensor.matmul(py[:], at[:, ib, :], w2t[hc][:, ib, :],
                                 start=(ib == 0), stop=(ib == IB - 1))
            yc = yp.tile([C, 512], bf16, tag="y", name="y", bufs=4)
            nc.scalar.activation(yc[:], py[:], AF.Copy)
            nc.scalar.dma_start(y_d[e, :, hc * 512:(hc + 1) * 512], yc[:])


def build_nc(reps=1):
    import concourse.bacc as bacc
    import concourse.mybir as mybir
    import concourse.tile as tile
    from concourse.masks import make_identity
    from contextlib import ExitStack

    f32 = mybir.dt.float32
    bf16 = mybir.dt.bfloat16

    nc = bacc.Bacc("TRN2", target_bir_lowering=False, debug=False,
                   num_devices=NCORES)

    xt_d = nc.dram_tensor("xt", [EPC, P, HB, C], bf16, kind="ExternalInput")
    w1t_d = nc.dram_tensor("w1t", [EPC, 4, P, 4 * I], bf16,
                           kind="ExternalInput")
    w3t_d = nc.dram_tensor("w3t", [EPC, 4, P, 4 * I], bf16,
                           kind="ExternalInput")
    w2t_d = nc.dram_tensor("w2t", [EPC, 4, P, IB * 512], bf16,
                           kind="ExternalInput")
    y_d = nc.dram_tensor("y", [EPC, C, H], bf16, kind="ExternalOutput")
    dram = (xt_d, w1t_d, w3t_d, w2t_d, y_d)

    with tile.TileContext(nc) as tc:
        with ExitStack() as ctx:
            pools = (
                ctx.enter_context(tc.tile_pool(name="xt", bufs=2)),
                ctx.enter_context(tc.tile_pool(name="w13", bufs=2)),
                ctx.enter_context(tc.tile_pool(name="w2", bufs=2)),
                ctx.enter_context(tc.tile_pool(name="a", bufs=2)),
                ctx.enter_context(tc.tile_pool(name="at", bufs=2)),
                ctx.enter_context(tc.tile_pool(name="y", bufs=2)),
                ctx.enter_context(tc.tile_pool(name="ps", bufs=1,
                                               space="PSUM")),
            )
            const = ctx.enter_context(tc.tile_pool(name="const", bufs=1))
            ident = const.tile([P, P], f32)
            make_identity(nc, ident[:])
            for _rep in range(reps):
                _emit_body(nc, mybir, pools, dram, ident)

    nc.compile()
    return nc


def _prep(hidden_states, gate_w, w1, w1_scale, w3, w3_scale, w2, w2_scale):
    """Host dispatch + weight prep. Returns (in_maps, routing meta)."""
    bf16 = _bf16()
    x = np.ascontiguousarray(hidden_states.reshape(T, H), dtype=np.float32)
    tids, cw = route(x, gate_w)
    in_maps = []
    for c in range(NCORES):
        lo = c * EPC
        xt_h = np.zeros((EPC, P, HB, C), dtype=bf16)
        w1t_h = np.empty((EPC, 4, P, 4 * I), dtype=bf16)
        w3t_h = np.empty((EPC, 4, P, 4 * I), dtype=bf16)
        w2t_h = np.empty((EPC, 4, P, IB * 512), dtype=bf16)
        for e in range(EPC):
            g = lo + e
            tid = tids[g]
            n = len(tid)
            assert n <= C, f"expert {g} load {n} > capacity {C}"
            xg = np.zeros((C, H), dtype=np.float32)
            xg[:n] = x[tid]
            # xT [H, C] -> [HB, 128, C] -> [128, HB, C]
            xt_h[e] = (xg.T.reshape(HB, P, C).transpose(1, 0, 2)
                       .astype(bf16))
            for (w, ws, out) in ((w1, w1_scale, w1t_h),
                                 (w3, w3_scale, w3t_h)):
                wd = (w[g].reshape(I, HB, P).astype(np.float32)
                      * ws[g][:, :, None]).reshape(I, H)
                # wT [H, I] -> per-chunk [4, 128, 4*I], contiguous/partition
                out[e] = (wd.T.reshape(4, 4, P, I).transpose(0, 2, 1, 3)
                          .reshape(4, P, 4 * I).astype(bf16))
            w2d = (w2[g].reshape(H, IB, P).astype(np.float32)
                   * w2_scale[g][:, :, None]).reshape(H, I)
            # w2T [I, H] -> per-h-chunk [4, 128, IB*512]
            w2t_h[e] = (w2d.T.reshape(IB, P, 4, 512).transpose(2, 1, 0, 3)
                        .reshape(4, P, IB * 512).astype(bf16))
        in_maps.append({
            "xt": np.ascontiguousarray(xt_h),
            "w1t": np.ascontiguousarray(w1t_h),
            "w3t": np.ascontiguousarray(w3t_h),
            "w2t": np.ascontiguousarray(w2t_h),
        })
    return in_maps, (tids, cw)


def shard_inputs(hidden_states, gate_w, w1, w1_scale, w3, w3_scale,
                 w2, w2_scale):
    in_maps, _ = _prep(np.asarray(hidden_states), np.asarray(gate_w),
                       np.asarray(w1), np.asarray(w1_scale),
                       np.asarray(w3), np.asarray(w3_scale),
                       np.asarray(w2), np.asarray(w2_scale))
    return in_maps


def kernel(hidden_states, gate_w, w1, w1_scale, w3, w3_scale, w2, w2_scale,
           top_k):
    global C
    assert int(top_k) == 2
    from concourse.bass_utils import run_bass_kernel_spmd

    hidden_states = np.asarray(hidden_states)
    B, S, _ = hidden_states.shape

    # capacity fallback: rebuild at C=128 if any expert exceeds 96 tokens
    x = np.ascontiguousarray(hidden_states.reshape(T, H), dtype=np.float32)
    max_load = max(len(t) for t in route(x, np.asarray(gate_w))[0])
    need_c = 96 if max_load <= 96 else 128
    assert max_load <= 128, f"expert load {max_load} > 128 unsupported"
    if need_c != C:
        C = need_c
        _CACHE.clear()
    if ("nc", C) not in _CACHE:
        _CACHE[("nc", C)] = build_nc()
    nc = _CACHE[("nc", C)]

    in_maps, (tids, cw) = _prep(hidden_states, np.asarray(gate_w),
                                np.asarray(w1), np.asarray(w1_scale),
                                np.asarray(w3), np.asarray(w3_scale),
                                np.asarray(w2), np.asarray(w2_scale))
    res = run_bass_kernel_spmd(nc, in_maps, list(range(NCORES)))
    y = np.zeros((T, H), dtype=np.float32)
    for c in range(NCORES):
        part = np.asarray(res.results[c]["y"]).astype(np.float32)
        for e in range(EPC):
            g = c * EPC + e
            tid = tids[g]
            n = len(tid)
            if n:
                y[tid] += cw[g][:, None] * part[e, :n]
    return np.ascontiguousarray(y).reshape(B, S, H).astype(np.float32)


# revision 20
# speedup vs baseline: 2.0256x; 2.0256x over previous
"""MiniMax-M2 sparse MoE block on 8 Trainium2 NeuronCores.

Expert-parallel with host-side dispatch (per the sharding hint): the host
computes the tiny router (512x16 GEMM + sigmoid + top-2), gathers each
expert's tokens, and ships per-core shards; each core runs its 2 local
experts' SwiGLU MLPs on just their routed tokens (capacity C=128 slots,
seed-stable max load ~84) and the host scatter-adds the expert outputs
back with the combine weights (the unshard step).

Weight prep happens at shard time on the host: block-dequant (the
reference's `_dequant`), cast to bf16, and transpose into the
contraction-major layout the PE needs. The device then streams ~20 MB of
bf16 weights per core -- the memory roofline this kernel targets -- and
does all the heavy math:

  - up/gate: stationary xT chunks [h,128 tok], moving w1T/w3T [h, I]
    slices, PSUM-accumulated over 16 h-blocks -> g,u [tok, I].
  - SwiGLU on ACT/DVE, PE-transpose of a -> aT [i, tok].
  - down: stationary aT chunks, moving w2T [i, H] slices, accumulated
    over 6 i-blocks -> y [tok, H], stored bf16.

All loads are a handful of large HWDGE DMAs (contiguous per partition).
"""

import os
import sys
import numpy as np

for _p in ("/opt/trn_rl_repo", "/root/.axon_site/_ro/trn_rl_repo"):
    if os.path.isdir(_p) and _p not in sys.path:
        sys.path.insert(0, _p)
        break

T, H, I, E = 512, 2048, 768, 16
NCORES, EPC = 8, 2
P = 128
HB, IB, TC = H // P, I // P, T // P      # 16, 6, 4
C = 96                                    # token capacity per expert
# (Binomial(512, 1/8) loads: mean 64, sd 7.5 -- 96 is a >4-sigma cap.
# kernel() falls back to a C=128 build if an expert ever exceeds it.)

_CACHE = {}


def _bf16():
    import ml_dtypes
    return ml_dtypes.bfloat16


def route(x, gate_w):
    """Host router: exact reference semantics (sigmoid + top-2 + sum-norm).

    Returns (tids, cw): per expert, the token ids routed to it and their
    combine weights.
    """
    logits = x.astype(np.float32) @ gate_w.T.astype(np.float32)
    s = (1.0 / (1.0 + np.exp(-logits))).astype(np.float32)
    order = np.argsort(-s, axis=-1, kind="stable")   # ties: lower idx first
    top2 = order[:, :2]
    denom = (np.take_along_axis(s, top2, axis=1).sum(axis=1)
             .astype(np.float32))
    tids, cw = [], []
    for e in range(E):
        hit = (top2 == e).any(axis=1)
        tid = np.nonzero(hit)[0]
        tids.append(tid)
        cw.append((s[tid, e] / denom[tid]).astype(np.float32))
    return tids, cw


def _emit_body(nc, mybir, pools, dram, ident):
    f32 = mybir.dt.float32
    bf16 = mybir.dt.bfloat16
    AF = mybir.ActivationFunctionType
    OP = mybir.AluOpType
    (xtp, w13p, w2p, ap, atp, yp, ps) = pools
    (xt_d, w1t_d, w3t_d, w2t_d, y_d) = dram

    for e in range(EPC):
        # ---- loads: chunk-contiguous HWDGE DMAs (128 descriptors each),
        # spread across both rings (sync + scalar) ----
        xt = xtp.tile([P, HB, C], bf16, tag="xt", name="xt")
        nc.sync.dma_start(xt[:], xt_d[e])
        w1t, w3t = [], []
        for k in range(4):
            t1 = w13p.tile([P, 4, I], bf16, tag="w1t", name="w1t", bufs=8)
            nc.sync.dma_start(t1[:], w1t_d[e, k])
            w1t.append(t1)
            t3 = w13p.tile([P, 4, I], bf16, tag="w3t", name="w3t", bufs=8)
            nc.scalar.dma_start(t3[:], w3t_d[e, k])
            w3t.append(t3)
        w2t = []
        for hc in range(4):
            t2 = w2p.tile([P, IB, 512], bf16, tag="w2t", name="w2t", bufs=8)
            (nc.sync if hc % 2 == 0 else nc.scalar).dma_start(
                t2[:], w2t_d[e, hc])
            w2t.append(t2)

        # ---- up/gate: psum [tok, i] accumulated over 16 h-blocks ----
        pg_lo = ps.tile([C, 512], f32, tag="pgl", name="pg_lo", bufs=1)
        pg_hi = ps.tile([C, 256], f32, tag="pgh", name="pg_hi", bufs=1)
        pu_lo = ps.tile([C, 512], f32, tag="pul", name="pu_lo", bufs=1)
        pu_hi = ps.tile([C, 256], f32, tag="puh", name="pu_hi", bufs=1)
        for hb in range(HB):
            st, sp = (hb == 0), (hb == HB - 1)
            k, j = hb // 4, hb % 4
            nc.tensor.matmul(pg_lo[:], xt[:, hb, :], w1t[k][:, j, 0:512],
                             start=st, stop=sp)
            nc.tensor.matmul(pg_hi[:], xt[:, hb, :], w1t[k][:, j, 512:768],
                             start=st, stop=sp)
            nc.tensor.matmul(pu_lo[:], xt[:, hb, :], w3t[k][:, j, 0:512],
                             start=st, stop=sp)
            nc.tensor.matmul(pu_hi[:], xt[:, hb, :], w3t[k][:, j, 512:768],
                             start=st, stop=sp)

        # ---- SwiGLU: a = silu(g) * u, f32 [tok, I] ----
        asb = ap.tile([C, I], f32, tag="a", name="a")
        for (pg, pu, lo, hi) in ((pg_lo, pu_lo, 0, 512),
                                 (pg_hi, pu_hi, 512, 768)):
            w = hi - lo
            sg = ap.tile([C, w], bf16, tag="sg%d" % lo, name="sg")
            nc.scalar.activation(sg[:], pg[:], AF.Sigmoid)
            xs = ap.tile([C, w], f32, tag="xs%d" % lo, name="xs")
            nc.vector.tensor_tensor(out=xs[:], in0=sg[:], in1=pg[:],
                                    op=OP.mult)
            nc.vector.tensor_tensor(out=asb[:, lo:hi], in0=xs[:], in1=pu[:],
                                    op=OP.mult)

        # ---- transpose a -> aT [i, tok] bf16 via PE ----
        at = atp.tile([P, IB, C], bf16, tag="at", name="at")
        for ic in range(IB):
            pt = ps.tile([P, C], f32, tag="pt", name="pt", bufs=2)
            nc.tensor.transpose(pt[:], asb[:, ic * P:(ic + 1) * P],
                                ident[0:C, 0:C])
            nc.vector.tensor_copy(at[:, ic, :], pt[:])

        # ---- down: psum [tok, h-chunk] accumulated over 6 i-blocks ----
        for hc in range(4):
            py = ps.tile([C, 512], f32, tag="py", name="py", bufs=2)
            for ib in range(IB):
                nc.tensor.matmul(py[:], at[:, ib, :], w2t[hc][:, ib, :],
                                 start=(ib == 0), stop=(ib == IB - 1))
            yc = yp.tile([C, 512], bf16, tag="y", name="y", bufs=4)
            nc.scalar.activation(yc[:], py[:], AF.Copy)
            nc.scalar.dma_start(y_d[e, :, hc * 512:(hc + 1) * 512], yc[:])


def build_nc(reps=1):
    import concourse.bacc as bacc
    import concourse.mybir as mybir
    import concourse.tile as tile
    from concourse.masks import make_identity
    from contextlib import ExitStack

    f32 = mybir.dt.float32
    bf16 = mybir.dt.bfloat16

    nc = bacc.Bacc("TRN2", target_bir_lowering=False, debug=False,
                   num_devices=NCORES)

    xt_d = nc.dram_tensor("xt", [EPC, P, HB, C], bf16, kind="ExternalInput")
    w1t_d = nc.dram_tensor("w1t", [EPC, 4, P, 4 * I], bf16,
                           kind="ExternalInput")
    w3t_d = nc.dram_tensor("w3t", [EPC, 4, P, 4 * I], bf16,
                           kind="ExternalInput")
    w2t_d = nc.dram_tensor("w2t", [EPC, 4, P, IB * 512], bf16,
                           kind="ExternalInput")
    y_d = nc.dram_tensor("y", [EPC, C, H], bf16, kind="ExternalOutput")
    dram = (xt_d, w1t_d, w3t_d, w2t_d, y_d)

    with tile.TileContext(nc) as tc:
        with ExitStack() as ctx:
            pools = (
                ctx.enter_context(tc.tile_pool(name="xt", bufs=2)),
                ctx.enter_context(tc.tile_pool(name="w13", bufs=2)),
                ctx.enter_context(tc.tile_pool(name="w2", bufs=2)),
                ctx.enter_context(tc.tile_pool(name="a", bufs=2)),
                ctx.enter_context(tc.tile_pool(name="at", bufs=2)),
                ctx.enter_context(tc.tile_pool(name="y", bufs=2)),
                ctx.enter_context(tc.tile_pool(name="ps", bufs=1,
                                               space="PSUM")),
            )
            const = ctx.enter_context(tc.tile_pool(name="const", bufs=1))
            ident = const.tile([P, P], f32)
            make_identity(nc, ident[:])
            for _rep in range(reps):
                _emit_body(nc, mybir, pools, dram, ident)

    nc.compile()
    return nc


def _prep(hidden_states, gate_w, w1, w1_scale, w3, w3_scale, w2, w2_scale):
    """Host dispatch + weight prep. Returns (in_maps, routing meta)."""
    bf16 = _bf16()
    x = np.ascontiguousarray(hidden_states.reshape(T, H), dtype=np.float32)
    tids, cw = route(x, gate_w)
    in_maps = []
    for c in range(NCORES):
        lo = c * EPC
        xt_h = np.zeros((EPC, P, HB, C), dtype=bf16)
        w1t_h = np.empty((EPC, 4, P, 4 * I), dtype=bf16)
        w3t_h = np.empty((EPC, 4, P, 4 * I), dtype=bf16)
        w2t_h = np.empty((EPC, 4, P, IB * 512), dtype=bf16)
        for e in range(EPC):
            g = lo + e
            tid = tids[g]
            n = len(tid)
            assert n <= C, f"expert {g} load {n} > capacity {C}"
            xg = np.zeros((C, H), dtype=np.float32)
            xg[:n] = x[tid]
            # xT [H, C] -> [HB, 128, C] -> [128, HB, C]
            xt_h[e] = (xg.T.reshape(HB, P, C).transpose(1, 0, 2)
                       .astype(bf16))
            for (w, ws, out) in ((w1, w1_scale, w1t_h),
                                 (w3, w3_scale, w3t_h)):
                wd = (w[g].reshape(I, HB, P).astype(np.float32)
                      * ws[g][:, :, None]).reshape(I, H)
                # wT [H, I] -> per-chunk [4, 128, 4*I], contiguous/partition
                out[e] = (wd.T.reshape(4, 4, P, I).transpose(0, 2, 1, 3)
                          .reshape(4, P, 4 * I).astype(bf16))
            w2d = (w2[g].reshape(H, IB, P).astype(np.float32)
                   * w2_scale[g][:, :, None]).reshape(H, I)
            # w2T [I, H] -> per-h-chunk [4, 128, IB*512]
            w2t_h[e] = (w2d.T.reshape(IB, P, 4, 512).transpose(2, 1, 0, 3)
                        .reshape(4, P, IB * 512).astype(bf16))
        in_maps.append({
            "xt": np.ascontiguousarray(xt_h),
            "w1t": np.ascontiguousarray(w1t_h),
            "w3t": np.ascontiguousarray(w3t_h),
            "w2t": np.ascontiguousarray(w2t_h),
        })
    return in_maps, (tids, cw)


def shard_inputs(hidden_states, gate_w, w1, w1_scale, w3, w3_scale,
                 w2, w2_scale):
    in_maps, _ = _prep(np.asarray(hidden_states), np.asarray(gate_w),
                       np.asarray(w1), np.asarray(w1_scale),
                       np.asarray(w3), np.asarray(w3_scale),
                       np.asarray(w2), np.asarray(w2_scale))
    return in_maps


def kernel(hidden_states, gate_w, w1, w1_scale, w3, w3_scale, w2, w2_scale,
           top_k):
    global C
    assert int(top_k) == 2
    from concourse.bass_utils import run_bass_kernel_spmd

    hidden_states = np.asarray(hidden_states)
    B, S, _ = hidden_states.shape

    # capacity fallback: rebuild at C=128 if any expert exceeds 96 tokens
    x = np.ascontiguousarray(hidden_states.reshape(T, H), dtype=np.float32)
    max_load = max(len(t) for t in route(x, np.asarray(gate_w))[0])
    need_c = 96 if max_load <= 96 else 128
    assert max_load <= 128, f"expert load {max_load} > 128 unsupported"
    if need_c != C:
        C = need_c
        _CACHE.clear()
    if ("nc", C) not in _CACHE:
        _CACHE[("nc", C)] = build_nc()
    nc = _CACHE[("nc", C)]

    in_maps, (tids, cw) = _prep(hidden_states, np.asarray(gate_w),
                                np.asarray(w1), np.asarray(w1_scale),
                                np.asarray(w3), np.asarray(w3_scale),
                                np.asarray(w2), np.asarray(w2_scale))
    res = run_bass_kernel_spmd(nc, in_maps, list(range(NCORES)))
    y = np.zeros((T, H), dtype=np.float32)
    for c in range(NCORES):
        part = np.asarray(res.results[c]["y"]).astype(np.float32)
        for e in range(EPC):
            g = c * EPC + e
            tid = tids[g]
            n = len(tid)
            if n:
                y[tid] += cw[g][:, None] * part[e, :n]
    return np.ascontiguousarray(y).reshape(B, S, H).astype(np.float32)
